# revision 70
# baseline (speedup 1.0000x reference)
"""Differential attention Trainium2 kernel (8 NeuronCores, SPMD).

Sharding: core c handles batch c//4, heads [4*(c%4), 4*(c%4)+4).

v2: software-pipelined around the ACT(exp) stream, which is the hard
floor (256 exp ACTIVATEs x ~1.15us). Structure:
  - Attention runs per (head, 512-query block): 16 t-steps, each one
    [128,1024] fp32 PSUM qk tile (2 maps x 512q) -> one exp ACTIVATE
    -> bf16 es -> 2 AV matmuls accumulating O^T_aug[65, 512] per map.
    PSUM: qk double-buffered (4 banks) + 2 po banks + 2 spare banks.
  - Projections: w stationary for q/k ([dims, tok] layout); v computed
    directly in [tok, dims] layout with x^T chunks stationary (no PE
    transposes). Pair-0 q/k + first v chunks form the ramp; remaining
    v chunks, pair-1 q/k, and the wo-phase for pair 0 are emitted as
    background thunks interleaved into the t-step stream so they run
    in the PE/DVE shadow of the ACT stream.
  - Epilogue (per head, per query block, on DVE + 8 PE transposes):
    neg_mu = -lam*r1/r2; pre = O1 + neg_mu*O2; rsqrt via bit trick +
    2 Newton steps; RMSNorm scale-invariance cancels 1/r1; norm_w*0.2
    folded into the wo-phase evacuation.
  - Final (replicates reference's cat(dim=1)->transpose->view quirk):
    F[d-block rows] = On_half[t,d].T @ wo contracting over 1024-token
    halves. Output rows disjoint across cores -> host scatter.
"""
import numpy as np

B, S, E, H = 2, 2048, 1024, 16
HD = 64
LAMBDA_INIT = 0.8
EPS = 1e-6

_CACHE = {}


def _build_nc():
    import concourse.bass as bass
    import concourse.tile as tile
    from concourse import bacc, mybir

    f32 = mybir.dt.float32
    bf = mybir.dt.bfloat16
    i32 = mybir.dt.int32
    i16 = mybir.dt.int16
    FT = mybir.ActivationFunctionType
    OP = mybir.AluOpType

    nc = bacc.Bacc("TRN2", target_bir_lowering=False, debug=False, num_devices=8)

    # xt layout: [p, (j, k, s)] — per-partition-contiguous 512-token
    # column blocks so each column-block DMA is a cheap dense pattern.
    xt_d = nc.dram_tensor("xt", [128, 4 * 8 * 512], bf, kind="ExternalInput").ap()
    wq_d = nc.dram_tensor("wq", [128, 8 * 768], bf, kind="ExternalInput").ap()
    wo_d = nc.dram_tensor("wo_full", [E, E], bf, kind="ExternalInput").ap()
    nw_d = nc.dram_tensor("nw", [2, 128, 1], f32, kind="ExternalInput").ap()
    lamq_d = nc.dram_tensor("lamq", [HD, 256], f32, kind="ExternalInput").ap()
    lamk_d = nc.dram_tensor("lamk", [HD, 2], f32, kind="ExternalInput").ap()
    id_d = nc.dram_tensor("ident", [128, 128], f32, kind="ExternalInput").ap()
    out_d = nc.dram_tensor("outp", [4, 2, HD, E], f32, kind="ExternalOutput").ap()

    MAGIC = 0x5F3759DF

    with tile.TileContext(nc) as tc:
        with (
            tc.tile_pool(name="consts", bufs=1) as consts,
            tc.tile_pool(name="qkv", bufs=1) as qkv,
            tc.tile_pool(name="vbuf", bufs=1) as vbuf,
            tc.tile_pool(name="onb", bufs=1) as onb,
            tc.tile_pool(name="pqk", bufs=2, space="PSUM") as pqk,
            tc.tile_pool(name="ppo", bufs=1, space="PSUM") as ppo,
            tc.tile_pool(name="psp", bufs=2, space="PSUM") as psp,
            tc.tile_pool(name="esb", bufs=8) as esb,
            tc.tile_pool(name="ep", bufs=2) as ep,
            tc.tile_pool(name="fe", bufs=2) as fe,
        ):
            # ---- DMAs: tiny consts first, then wq/xt interleaved per
            # e-chunk so the ramp's k-loop can start on chunk 0 early.
            scr = consts.tile([128, 2], f32)
            nc.vector.memset(scr[:, 0:1], 0.0)
            wq_sb = consts.tile([128, 8 * 768], bf)
            nc.sync.dma_start(wq_sb[:], wq_d[:])
            wqv = wq_sb.rearrange("p (k c) -> p k c", k=8)
            xt_sb = consts.tile([128, 8 * 2048], bf)
            xtv = xt_sb.rearrange("p (j k s) -> p j k s", j=4, k=8)
            for j in range(2):
                # issue on the (ramp-idle) ACT queue, parallel to sync
                nc.scalar.dma_start(
                    xt_sb[:, 4096 * j : 4096 * (j + 1)],
                    xt_d[:, 4096 * j : 4096 * (j + 1)],
                )
            nc.scalar.activation(scr[:, 1:2], scr[:, 0:1], FT.Exp)  # table warmup
            for j in range(2, 4):
                nc.scalar.dma_start(
                    xt_sb[:, 4096 * j : 4096 * (j + 1)],
                    xt_d[:, 4096 * j : 4096 * (j + 1)],
                )
            lamq_sb = consts.tile([HD, 256], f32)
            nc.sync.dma_start(lamq_sb[:], lamq_d[:])
            lamk_sb = consts.tile([HD, 2], f32)
            nc.sync.dma_start(lamk_sb[:], lamk_d[:])
            nw_sb = consts.tile([128, 2], f32)
            nc.sync.dma_start(nw_sb[:], nw_d.rearrange("p2 r one -> r (p2 one)"))
            id_sb = consts.tile([128, 128], f32)
            nc.sync.dma_start(id_sb[:], id_d[:])
            wo_sb = consts.tile([128, 8 * E], bf)
            nc.sync.dma_start(
                wo_sb.rearrange("p (k j) -> p k j", k=8),
                wo_d.rearrange("(k p) j -> p k j", p=128),
            )
            wov = wo_sb.rearrange("p (k j) -> p k j", k=8)
            el = consts.tile([128, 2], f32)
            lam_bc = consts.tile([128, 1], f32)

            # persistent qkv^T tiles per pair: rows [hA q1|q2, hB q1|q2]
            QT = [qkv.tile([128, S], bf, name=f"QT{p}") for p in range(2)]
            KT = [qkv.tile([128, S], bf, name=f"KT{p}") for p in range(2)]
            # V_aug per head: 16 chunks of [128 tok, 65] (col 64 = ones)
            VA = [vbuf.tile([128, 16 * 65], bf, name=f"VA{h}") for h in range(4)]
            for h in range(4):
                nc.vector.memset(
                    VA[h].rearrange("p (t c) -> p t c", c=65)[:, :, 64:65], 1.0
                )
            # On per head: [q, d] layout, 16 chunks of [128 tok, 64]
            ON = [onb.tile([128, 16 * HD], bf, name=f"ON{h}") for h in range(4)]

            # ---- emission helpers ----
            prstate = {}

            def emit_qk_proj_a(p, j, m):
                # m: 0=q, 1=k. acc[dims 128, 512 tok] = W^T x, accumulate e.
                off = 256 * p + 128 * m
                acc = psp.tile([128, 512], f32, tag="sp", name="acc")
                prstate[(p, j, m)] = acc
                for k in range(4):
                    nc.tensor.matmul(
                        acc[:],
                        wqv[:, k, off : off + 128],
                        xtv[:, j, k, :],
                        start=(k == 0),
                        stop=False,
                    )

            def emit_qk_proj_b(p, j, m):
                off = 256 * p + 128 * m
                acc = prstate.pop((p, j, m))
                for k in range(4, 8):
                    nc.tensor.matmul(
                        acc[:],
                        wqv[:, k, off : off + 128],
                        xtv[:, j, k, :],
                        start=False,
                        stop=(k == 7),
                    )
                dst = QT[p] if m == 0 else KT[p]
                nc.vector.tensor_copy(dst[:, 512 * j : 512 * (j + 1)], acc[:])

            def emit_qk_proj(p, j, m):
                emit_qk_proj_a(p, j, m)
                emit_qk_proj_b(p, j, m)

            def emit_v(tc_idx, pr):
                # direct [tok, v] projection: x^T chunk stationary.
                # Fresh sp tile per chunk: a tile must collect ALL its
                # instructions before later same-tag allocations occur.
                vd = psp.tile([128, 512], f32, tag="sp", name="vd")
                jj, off = tc_idx // 4, 128 * (tc_idx % 4)
                for k in range(8):
                    nc.tensor.matmul(
                        vd[:, 0:128],
                        xtv[:, jj, k, off : off + 128],
                        wqv[:, k, 512 + 128 * pr : 640 + 128 * pr],
                        start=(k == 0),
                        stop=(k == 7),
                    )
                for hh in range(2):
                    nc.vector.tensor_copy(
                        VA[2 * pr + hh][:, 65 * tc_idx : 65 * tc_idx + 64],
                        vd[:, 64 * hh : 64 * (hh + 1)],
                    )

            def emit_qk_exp(h, jq, t, dve=False):
                p, hh = h // 2, h % 2
                qk = pqk.tile([128, 1024], f32, tag="qk")
                for m in range(2):
                    u = 2 * hh + m
                    nc.tensor.matmul(
                        qk[:, 512 * m : 512 * (m + 1)],
                        KT[p][32 * u : 32 * (u + 1), 128 * t : 128 * (t + 1)],
                        QT[p][32 * u : 32 * (u + 1), 512 * jq : 512 * (jq + 1)],
                        start=True,
                        stop=True,
                        tile_position=(32 * u, 0),
                    )
                es = esb.tile([128, 1024], bf, tag="es")
                if dve:
                    # Schraudolph bf16 exp on DVE: bits(exp(x/8)) ~
                    # i16(A*x + B). Offloads the saturated ACT engine.
                    nc.vector.tensor_scalar(
                        es.bitcast(i16)[:], qk[:], 23.0831146, 16248.6319,
                        op0=OP.mult, op1=OP.add,
                    )
                else:
                    nc.scalar.activation(es[:], qk[:], FT.Exp, scale=0.125)
                return es

            def emit_ep_copy(po):
                osb = []
                for m in range(2):
                    t_ = ep.tile([65, 512], f32, tag=f"osb{m}", name=f"osb{m}")
                    nc.vector.tensor_copy(t_[:], po[m][:])
                    osb.append(t_)
                return osb

            def emit_ep_rest(h, jq, osb):
                ot = []
                for m in range(2):
                    trb = psp.tile([128, 512], f32, tag="sp", name="trb")
                    for v in range(4):
                        nc.tensor.transpose(
                            trb[:, 65 * v : 65 * (v + 1)],
                            osb[m][0:65, 128 * v : 128 * (v + 1)],
                            id_sb[0:65, 0:65],
                        )
                    o_t = ep.tile([128, 260], f32, tag=f"ot{m}", name=f"ot{m}")
                    nc.vector.tensor_copy(o_t[:], trb[:, 0:260])
                    ot.append(o_t)
                rv = [o.rearrange("p (v c) -> p v c", c=65)[:, :, 64:65] for o in ot]
                rcp = ep.tile([128, 4], f32, tag="rcp")
                nc.vector.reciprocal(rcp[:], rv[1][:, :, 0])
                nmu = ep.tile([128, 4], f32, tag="nmu")
                nc.vector.scalar_tensor_tensor(
                    nmu[:], rcp[:], lam_bc[:], rv[0][:, :, 0], op0=OP.mult, op1=OP.mult
                )
                o1r = ot[0].rearrange("p (v c) -> p v c", c=65)
                o2r = ot[1].rearrange("p (v c) -> p v c", c=65)
                pre = ep.tile([128, 256], f32, tag="pre")
                for v in range(4):
                    nc.vector.scalar_tensor_tensor(
                        pre[:, HD * v : HD * (v + 1)],
                        o2r[:, v, 0:HD],
                        nmu[:, v : v + 1],
                        o1r[:, v, 0:HD],
                        op0=OP.mult,
                        op1=OP.add,
                    )
                sqs = ep.tile([128, 256], f32, tag="sqs")
                nc.vector.tensor_tensor(sqs[:], pre[:], pre[:], op=OP.mult)
                ss = ep.tile([128, 4], f32, tag="ss")
                for v in range(4):
                    nc.vector.tensor_reduce(
                        ss[:, v : v + 1],
                        sqs[:, HD * v : HD * (v + 1)],
                        axis=mybir.AxisListType.X,
                        op=OP.add,
                    )
                # rsqrt(ss/64 + eps) via bit trick + 2 Newton steps
                msc = ep.tile([128, 4], f32, tag="msc")
                nc.vector.tensor_scalar(
                    msc[:], ss[:], 1.0 / HD, EPS, op0=OP.mult, op1=OP.add
                )
                y0i = ep.tile([128, 4], i32, tag="y0i")
                nc.vector.tensor_scalar(
                    y0i[:], msc.bitcast(i32)[:], 1, None, op0=OP.arith_shift_right
                )
                nc.vector.tensor_scalar(
                    y0i[:], y0i[:], -1, MAGIC, op0=OP.mult, op1=OP.add
                )
                y = y0i.bitcast(f32)
                t1 = ep.tile([128, 4], f32, tag="t1")
                for _ in range(1):
                    nc.vector.tensor_tensor(t1[:], y[:], y[:], op=OP.mult)
                    nc.vector.tensor_tensor(t1[:], t1[:], msc[:], op=OP.mult)
                    nc.vector.tensor_scalar(
                        t1[:], t1[:], -0.5, 1.5, op0=OP.mult, op1=OP.add
                    )
                    nc.vector.tensor_tensor(y[:], y[:], t1[:], op=OP.mult)
                for v in range(4):
                    c_idx = 4 * jq + v
                    nc.vector.tensor_scalar(
                        ON[h][:, HD * c_idx : HD * (c_idx + 1)],
                        pre[:, HD * v : HD * (v + 1)],
                        y[:, v : v + 1],
                        None,
                        op0=OP.mult,
                    )

            def emit_wo_mm(p, thi, n, fp, k0, k1):
                for k in range(k0, k1):
                    c_idx = 8 * thi + k
                    nc.tensor.matmul(
                        fp[0:64, :],
                        ON[2 * p][:, HD * c_idx : HD * (c_idx + 1)],
                        wov[:, k, 512 * n : 512 * (n + 1)],
                        start=(k == 0),
                        stop=(k == 7),
                        tile_position=(0, 0),
                    )
                    nc.tensor.matmul(
                        fp[64:128, :],
                        ON[2 * p + 1][:, HD * c_idx : HD * (c_idx + 1)],
                        wov[:, k, 512 * n : 512 * (n + 1)],
                        start=(k == 0),
                        stop=(k == 7),
                        tile_position=(0, 64),
                    )

            def emit_wo_out(p, thi, n, fp):
                fsb = fe.tile([128, 512], f32, tag="fsb")
                nc.vector.tensor_scalar(
                    fsb[:], fp[:], nw_sb[:, p : p + 1], None, op0=OP.mult
                )
                nc.sync.dma_start(
                    out_d[2 * p, thi, :, 512 * n : 512 * (n + 1)], fsb[0:64, :]
                )
                nc.sync.dma_start(
                    out_d[2 * p + 1, thi, :, 512 * n : 512 * (n + 1)], fsb[64:128, :]
                )

            def make_wo_groups(p, thi):
                groups = []
                for n in range(2):
                    st = {}

                    def t1(p=p, thi=thi, n=n, st=st):
                        st["fp"] = psp.tile([128, 512], f32, tag="sp", name="fp")
                        emit_wo_mm(p, thi, n, st["fp"], 0, 4)

                    def t2(p=p, thi=thi, n=n, st=st):
                        emit_wo_mm(p, thi, n, st["fp"], 4, 8)
                        emit_wo_out(p, thi, n, st["fp"])

                    groups.append([t1, t2])
                return groups

            # ---- ramp ----
            # ---- ramp: only what the first t-steps need — k/q for the
            # first token block + first two v chunks. k(j1..3) stream in
            # inside block 0 ahead of their t=4j read deadlines; q(j>0)
            # and everything for pair 1 are background thunks.
            emit_qk_proj(0, 0, 1)
            emit_qk_proj(0, 0, 0)
            emit_v(0, 0)
            emit_v(1, 0)
            # lambda scalar (replicated to all 128 partitions)
            psl = psp.tile([128, 512], f32, tag="sp")
            nc.tensor.matmul(
                psl[:, 0:1], lamq_sb[:, 0:128], lamk_sb[:, 0:1], start=True, stop=True
            )
            nc.tensor.matmul(
                psl[:, 1:2], lamq_sb[:, 128:256], lamk_sb[:, 1:2], start=True, stop=True
            )
            nc.scalar.activation(el[:], psl[:, 0:2], FT.Exp)
            # lam_bc = (e2 - 0.8) - e1 = -(e1 - e2 + 0.8)
            nc.vector.scalar_tensor_tensor(
                lam_bc[:], el[:, 1:2], -LAMBDA_INIT, el[:, 0:1],
                op0=OP.add, op1=OP.subtract,
            )

            # ---- background queue: groups of thunks; a group's sp-pool
            # tiles live only within the group, and groups are popped at
            # steps where no other sp allocations can interleave.
            bg = []
            for j in range(2, 4):
                bg.append([
                    lambda j=j: emit_qk_proj_a(0, j, 0),
                    lambda j=j: emit_qk_proj_b(0, j, 0),
                ])
            for j in range(4):
                for m in range(2):
                    bg.append([
                        lambda j=j, m=m: emit_qk_proj_a(1, j, m),
                        lambda j=j, m=m: emit_qk_proj_b(1, j, m),
                    ])
            for tc_i in range(0, 16, 2):
                bg.append([
                    lambda tc_i=tc_i: emit_v(tc_i, 1),
                    lambda tc_i=tc_i: emit_v(tc_i + 1, 1),
                ])

            # ---- attention blocks ----
            cur_grp = []

            def pop_bg(t):
                # one thunk per odd step; only start a group that fits
                # in this block's remaining pop slots (a group's sp
                # tiles must not stay live across an epilogue emission).
                if cur_grp:
                    cur_grp.pop(0)()
                    return
                slots = (13 - t) // 2 + 1
                if bg and len(bg[0]) <= slots:
                    cur_grp.extend(bg.pop(0))
                    cur_grp.pop(0)()

            pending_ep = None
            for h in range(4):
                for jq in range(4):
                    first = h == 0 and jq == 0
                    if (h, jq) == (1, 3):
                        bg += make_wo_groups(0, 0)
                    elif (h, jq) == (2, 1):
                        bg += make_wo_groups(0, 1)
                    elif (h, jq) == (3, 3):
                        bg += make_wo_groups(1, 0)
                    es_q = [emit_qk_exp(h, jq, 0), emit_qk_exp(h, jq, 1)]
                    po = [
                        ppo.tile([65, 512], f32, tag=f"po{m}", name=f"po{m}")
                        for m in range(2)
                    ]
                    osb_p = None
                    for t in range(16):
                        if t < 14:
                            es_q.append(
                                emit_qk_exp(
                                    h, jq, t + 2,
                                    dve=not first and (t + 2) in (5, 7, 11, 13),
                                )
                            )
                        if t == 1 and pending_ep is not None:
                            osb_p = (pending_ep[0], pending_ep[1],
                                     emit_ep_copy(pending_ep[2]))
                            pending_ep = None
                        elif t == 14 and osb_p is not None:
                            emit_ep_rest(*osb_p)
                            osb_p = None
                        es_t = es_q.pop(0)
                        for m in range(2):
                            nc.tensor.matmul(
                                po[m][:],
                                VA[h][:, 65 * t : 65 * (t + 1)],
                                es_t[:, 512 * m : 512 * (m + 1)],
                                start=(t == 0),
                                stop=(t == 15),
                            )
                        if first:
                            # JIT schedule: k(j) chains before their
                            # t=4j reads; v chunks before their AV step;
                            # q(j1) early so the next block isn't starved.
                            if t in (0, 4, 8):
                                emit_qk_proj(0, t // 4 + 1, 1)
                            elif t in (1, 5, 9):
                                emit_v(t + 1, 0)
                                emit_v(t + 2, 0)
                            elif t < 14:
                                emit_v(t + 2, 0)
                            elif t == 14:
                                emit_qk_proj_a(0, 1, 0)
                            else:
                                emit_qk_proj_b(0, 1, 0)
                        elif t % 2 == 1 and t >= 3:
                            pop_bg(t)
                    pending_ep = (h, jq, po)
            emit_ep_rest(pending_ep[0], pending_ep[1], emit_ep_copy(pending_ep[2]))
            # warm-keeper: keep HAM at full clock through the final
            # epilogue's serial DVE chain so the wo matmuls run warm.
            dum = ppo.tile([65, 512], f32, tag="po0", name="dum")
            for _w in range(14):
                nc.tensor.matmul(
                    dum[:], VA[0][:, 0:65], xt_sb[:, 0:512], start=True, stop=True
                )

            # ---- drain: remaining background + last wo quarter ----
            for grp in bg:
                for th in grp:
                    th()
            for grp in make_wo_groups(1, 1):
                for th in grp:
                    th()

    nc.compile()
    return nc


def _shard_inputs(x, w_qkv, wo, lambda_q1, lambda_q2, lambda_k1, lambda_k2, norm_w):
    import ml_dtypes

    bf16 = ml_dtypes.bfloat16
    x = np.asarray(x, dtype=np.float32)
    w_qkv = np.asarray(w_qkv, dtype=np.float32)
    wo_b = np.asarray(wo, dtype=np.float32).astype(bf16)
    norm_w = np.asarray(norm_w, dtype=np.float32)
    lq1 = np.asarray(lambda_q1, np.float32)
    lq2 = np.asarray(lambda_q2, np.float32)
    lk1 = np.asarray(lambda_k1, np.float32)
    lk2 = np.asarray(lambda_k2, np.float32)

    lamq = np.concatenate(
        [np.repeat(lq1[:, None], 128, axis=1), np.repeat(lq2[:, None], 128, axis=1)],
        axis=1,
    ).astype(np.float32)  # [64, 256]
    lamk = np.stack([lk1, lk2], axis=1).astype(np.float32)  # [64, 2]
    ident = np.eye(128, dtype=np.float32)

    in_maps = []
    for c in range(8):
        beta, g = divmod(c, 4)
        heads = [4 * g + i for i in range(4)]
        # [p, (j, k, s)]: xt2[p, j, k, c] = x^T[128k+p, 512j+c]
        xt = np.ascontiguousarray(
            x[beta].T.reshape(8, 128, 4, 512).transpose(1, 2, 0, 3).reshape(128, 16384)
        ).astype(bf16)
        # wq layout per e-chunk k: [q-pair0 128 | k-pair0 128 | q-pair1
        # 128 | k-pair1 128 | v all-heads 256]; within each 128: hA|hB.
        wq2 = np.empty((128, 8, 768), np.float32)
        for k in range(8):
            rows = w_qkv[128 * k : 128 * (k + 1), :]  # [128, 3072]
            for p in range(2):
                hA, hB = heads[2 * p], heads[2 * p + 1]
                base = 256 * p
                wq2[:, k, base + 0 : base + 64] = rows[:, 64 * hA : 64 * hA + 64]
                wq2[:, k, base + 64 : base + 128] = rows[:, 64 * hB : 64 * hB + 64]
                wq2[:, k, base + 128 : base + 192] = rows[:, E + 64 * hA : E + 64 * hA + 64]
                wq2[:, k, base + 192 : base + 256] = rows[:, E + 64 * hB : E + 64 * hB + 64]
            for i, hh in enumerate(heads):
                wq2[:, k, 512 + 64 * i : 512 + 64 * (i + 1)] = rows[
                    :, 2 * E + 64 * hh : 2 * E + 64 * hh + 64
                ]
        wq = wq2.reshape(128, 8 * 768)
        nw = np.empty((2, 128, 1), np.float32)
        for p in range(2):
            nw[p, 0:64, 0] = norm_w[heads[2 * p]] * (1.0 - LAMBDA_INIT)
            nw[p, 64:128, 0] = norm_w[heads[2 * p + 1]] * (1.0 - LAMBDA_INIT)
        in_maps.append(
            {
                "xt": xt,
                "wq": wq.astype(bf16),
                "wo_full": wo_b,
                "nw": nw,
                "lamq": lamq,
                "lamk": lamk,
                "ident": ident,
            }
        )
    return in_maps


def kernel(**inputs):
    from concourse import bass_utils

    if "nc" not in _CACHE:
        _CACHE["nc"] = _build_nc()
    nc = _CACHE["nc"]

    in_maps = _shard_inputs(**inputs)
    res = bass_utils.run_bass_kernel_spmd(nc, in_maps, core_ids=list(range(8)))

    out = np.zeros((B, S, E), np.float32)
    for c in range(8):
        beta, g = divmod(c, 4)
        part = res.results[c]["outp"]  # [4, 2, 64, 1024] = [h_l, t_hi, d, j]
        ob = out[beta].reshape(HD, 32, E)  # s' = 32*d + 2*h + t_hi
        for hl in range(4):
            h = 4 * g + hl
            for thi in range(2):
                ob[:, 2 * h + thi, :] += part[hl, thi]
    return out
